# revision 3
# baseline (speedup 1.0000x reference)
"""Trainium2 Bass kernel: Gaussian-splat covariance from (scaling, rotation).

Reference math per point:
  s = sigmoid(scaling)*(SMAX-SMIN) + SMIN   (SMIN dropped: rel err < 2e-3)
  q = rotation/|rotation|; R = quat matrix; L = R diag(s); C = L L^T
  out = upper-tri 6 of C.

Rank-2 trick: R orthogonal => C = R diag(w) R^T with w_j = s_j^2 satisfies
  C = (w0-w2) r0 r0^T + (w1-w2) r1 r1^T + w2 I      (r_j = j-th column of R)
Only columns 0,1 of R are needed. With unnormalized quaternion columns
c_j = n2 * r_j (n2 = |q|^2) and s_j = A*sig_j:
  C = U0 * G0 + U1 * G1 + w2 I,   G_j = c_j c_j^T,
  U_j = (SIGSQA_j - SIGSQA_2) * EXPH,  SIGSQA = A*sig^2,  EXPH = A/n2^2,
  w2 = A * SIGSQA_2.
Column entries (a=r^2,b=x^2,c=y^2,d=z^2):
  c_0 = (D0, E3, E5) = (n2-2(c+d), 2xy+2rz, 2xz-2ry)
  c_1 = (E1, D1, E6) = (2xy-2rz, n2-2(b+d), 2yz+2rx)

fp16 end-to-end, component-plane layout: host sends [7, Pc] per core
(plane order r,z,y,x,s0,s1,s2), kernel emits [6, Pc] rows
(C00,C11,C22,C01,C02,C12); host reorders/casts. All DVE ops contiguous
plane ops in 2x fp16 mode (tensor_scalar 4x); ACT does squares/sigmoid/
ln/exp. Software-pipelined: the final assembly of tile t-1 is emitted
during tile t so it never stalls on tile t-1's ACT squares.
"""

import os

import numpy as np

import concourse.bass as bass
import concourse.mybir as mybir
from concourse.tile import TileContext

F16 = mybir.dt.float16
F32 = mybir.dt.float32
ALU = mybir.AluOpType
ACTF = mybir.ActivationFunctionType

SCALE_MIN = 1e-4
SCALE_MAX = 10.0
A_SC = SCALE_MAX - SCALE_MIN
LN_A = float(np.log(A_SC))
SQRT_A = float(np.sqrt(A_SC))

N_CORES = 8
N_TOTAL = 4_000_000

F_PTS = 1000
T_TILES = 4
P_CORE = 128 * F_PTS * T_TILES  # 512000/core; x8 = 4.096M >= 4M


def _split_sync_waits(nc, nop_max=1):
    """This container's walrus encodes at most 2 sync waits per instruction
    (and none on Drain). Move excess waits onto dedicated NoOps upstream."""
    n = 0
    for bb in nc.main_func.blocks:
        out = []
        for ins in bb.instructions:
            si = ins.sync_info
            waits = list(si.on_wait) if (si is not None and si.on_wait) else []
            is_drain = type(ins).__name__ == "InstDrain"
            limit = 0 if is_drain and len(waits) > 1 else 1
            if len(waits) > limit:
                keep = waits[-limit:] if limit else []
                extra = waits[:-limit] if limit else waits
                for i0 in range(0, len(extra), nop_max):
                    n += 1
                    nop = mybir.InstNoOp(name=f"waitsplit_{n}", ins=[], outs=[])
                    nop.engine = ins.engine
                    nop.sync_info = mybir.SyncInfo(
                        on_wait=extra[i0 : i0 + nop_max], on_update=[]
                    )
                    out.append(nop)
                ins.sync_info = mybir.SyncInfo(
                    on_wait=keep, on_update=list(si.on_update or [])
                )
            out.append(ins)
        bb.instructions[:] = out
    return n


def build_nc(F=F_PTS, T=T_TILES):
    nc = bass.Bass()
    P = 128
    npts = P * F * T

    in_d = nc.declare_dram_parameter("in7", [7, npts], F16, isOutput=False)
    out_d = nc.declare_dram_parameter("out6", [6, npts], F16, isOutput=True)
    in_r = in_d[:, :].rearrange("c (t p f) -> t p c f", p=P, f=F)
    out_r = out_d[:, :].rearrange("c (t p f) -> t p c f", p=P, f=F)

    ve = nc.vector
    act = nc.scalar

    def pl(tile, i, n=1, s=1):
        r = tile[:].rearrange("p (k f) -> p k f", f=F)
        if n == 1:
            return r[:, i : i + 1]
        stop = i + (n - 1) * s + 1 if s > 0 else (i + (n - 1) * s - 1)
        if s < 0 and stop < 0:
            stop = None
        return r[:, i:stop:s]

    def bk(tile, i, n):
        r = tile[:].rearrange("p (k f) -> p k f", f=F)
        return r[:, i : i + 1].broadcast_to((P, n, F))

    with TileContext(nc) as tc:
        with nc.allow_low_precision(reason="fp16 kernel, tol 2e-2"), \
             tc.tile_pool(name="cst", bufs=1) as cst, \
             tc.tile_pool(name="io", bufs=2) as io, \
             tc.tile_pool(name="wk", bufs=1) as wk, \
             tc.tile_pool(name="pp", bufs=2) as pp:

            CONST = cst.tile([P, 4], F32, tag="const")
            ve.memset(CONST[:, 0:1], -1.0)     # exp scale (inv = 1/n2)
            ve.memset(CONST[:, 1:2], 0.0)      # exp bias
            ve.memset(CONST[:, 2:3], A_SC)     # square scale: SGA = A^2 sig^2

            prev = None  # state for software-pipelined tail of tile t-1

            def emit_tail(st):
                """Assembly of a finished tile: C = U0*G0 + U1*G1 + w2 I."""
                G, UW, W2, OUT, t_idx = st
                M1 = wk.tile([P, 6 * F], F16, tag="m1")
                M2 = wk.tile([P, 6 * F], F16, tag="m2")
                ve.tensor_tensor(
                    M1[:].rearrange("p (k f) -> p k f", f=F),
                    bk(UW, 0, 6), pl(G, 0, 6), ALU.mult)
                ve.tensor_tensor(
                    M2[:].rearrange("p (k f) -> p k f", f=F),
                    bk(UW, 1, 6), pl(G, 6, 6), ALU.mult)
                # off-diagonals: OUT[3:6] = M1[3:6] + M2[3:6]
                ve.tensor_tensor(pl(OUT, 3, 3), pl(M1, 3, 3), pl(M2, 3, 3), ALU.add)
                # diagonals: OUT[0:3] = (M1[0:3] + M2[0:3]) + w2  (DG in M2[3:6])
                ve.tensor_tensor(pl(M2, 3, 3), pl(M1, 0, 3), pl(M2, 0, 3), ALU.add)
                ve.tensor_tensor(pl(OUT, 0, 3), pl(M2, 3, 3), bk(W2, 0, 3), ALU.add)
                nc.sync.dma_start(out_r[t_idx], OUT[:].rearrange("p (c f) -> p c f", f=F))

            for t in range(T):
                IN = io.tile([P, 7 * F], F16, tag="in")
                OUT = io.tile([P, 6 * F], F16, tag="out")
                nc.sync.dma_start(IN[:].rearrange("p (c f) -> p c f", f=F), in_r[t])

                SCR = wk.tile([P, 6 * F], F16, tag="scr")  # SQ (a d c b) then PRD
                if os.environ.get("NOSHARE"):
                    SQ = wk.tile([P, 4 * F], F16, tag="sq4")
                else:
                    SQ = SCR
                SM = wk.tile([P, 4 * F], F16, tag="sm")    # cd bd ab n2
                T1 = wk.tile([P, 2 * F], F16, tag="t1")    # ln(n2), EXPH=A/n2^2
                DT = wk.tile([P, 2 * F], F16, tag="dt")    # -2cd -2bd
                SIG = wk.tile([P, 3 * F], F16, tag="sig")
                SGA = wk.tile([P, 3 * F], F16, tag="sga")  # A*sig^2
                P2 = wk.tile([P, 3 * F], F16, tag="p2")    # 2r 2y 2x
                PRD = SCR                                  # xy2 xz2 yz2 rz2 ry2 rx2
                COL = wk.tile([P, 6 * F], F16, tag="col")  # D0 E3 E5 | E1 D1 E6
                G = pp.tile([P, 12 * F], F16, tag="g")     # G0(6), G1(6): d00 d11 d22 o01 o02 o12
                UW = pp.tile([P, 2 * F], F16, tag="uw")    # U0 U1
                W2 = pp.tile([P, 1 * F], F16, tag="w2")

                # ACT chain (sigmoid set, then ln/exp set; squares ride anywhere)
                act.activation(SQ[:, : 4 * F], IN[:, : 4 * F], ACTF.Square)
                act.activation(SIG[:], IN[:, 4 * F :], ACTF.Sigmoid)
                if os.environ.get("NOSQSCALE"):
                    act.activation(SGA[:], SIG[:], ACTF.Square)
                    ve.tensor_scalar_mul(SGA[:], SGA[:], A_SC)
                else:
                    act.activation(SGA[:], SIG[:], ACTF.Square, scale=CONST[:, 2:3])

                # n2 block: SM = (cd, bd, ab, n2)
                ve.tensor_tensor(pl(SM, 0, 2), pl(SQ, 2, 2), bk(SQ, 1, 2), ALU.add)
                ve.tensor_tensor(pl(SM, 2), pl(SQ, 0), pl(SQ, 3), ALU.add)
                ve.tensor_tensor(pl(SM, 3), pl(SM, 2), pl(SM, 0), ALU.add)

                # inv = 1 / n2
                act.activation(pl(T1, 0), pl(SM, 3), ACTF.Ln)
                act.activation(pl(T1, 1), pl(T1, 0), ACTF.Exp,
                               scale=CONST[:, 0:1], bias=CONST[:, 1:2])

                # normalized diagonal entries: Dhat = 1 - 2*(cd,bd)*inv
                ve.tensor_tensor(DT[:].rearrange("p (k f) -> p k f", f=F),
                                 pl(SM, 0, 2), bk(T1, 1, 2), ALU.mult)
                if os.environ.get("NOPAIR"):
                    ve.tensor_scalar(pl(COL, 0), pl(DT, 0), -2.0, 1.0, ALU.mult, ALU.add)
                    ve.tensor_scalar(pl(COL, 4), pl(DT, 1), -2.0, 1.0, ALU.mult, ALU.add)
                else:
                    ve.tensor_scalar(pl(COL, 0, 2, 4),
                                     DT[:].rearrange("p (k f) -> p k f", f=F),
                                     -2.0, 1.0, ALU.mult, ALU.add)

                # doubled+normalized products: P2 = 2*(r,y,x)/n2
                ve.tensor_scalar_mul(pl(P2, 0), pl(IN, 0), 2.0)        # 2r
                ve.tensor_scalar_mul(pl(P2, 1, 2), pl(IN, 2, 2), 2.0)  # 2y 2x
                ve.tensor_tensor(P2[:].rearrange("p (k f) -> p k f", f=F),
                                 P2[:].rearrange("p (k f) -> p k f", f=F),
                                 bk(T1, 1, 3), ALU.mult)
                ve.tensor_tensor(pl(PRD, 0), pl(P2, 2), pl(IN, 2), ALU.mult)  # xy2
                ve.tensor_tensor(pl(PRD, 1), pl(P2, 2), pl(IN, 1), ALU.mult)  # xz2
                ve.tensor_tensor(pl(PRD, 2), pl(P2, 1), pl(IN, 1), ALU.mult)  # yz2
                ve.tensor_tensor(pl(PRD, 3, 3), bk(P2, 0, 3), pl(IN, 1, 3), ALU.mult)  # rz2 ry2 rx2

                # E entries: E3@1 = xy2+rz2, E6@5 = yz2+rx2 (one strided pair)
                if os.environ.get("NOPAIR"):
                    ve.tensor_tensor(pl(COL, 1), pl(PRD, 0), pl(PRD, 3), ALU.add)
                    ve.tensor_tensor(pl(COL, 5), pl(PRD, 2), pl(PRD, 5), ALU.add)
                else:
                    ve.tensor_tensor(pl(COL, 1, 2, 4), pl(PRD, 0, 2, 2), pl(PRD, 3, 2, 2), ALU.add)
                ve.tensor_tensor(pl(COL, 3), pl(PRD, 0), pl(PRD, 3), ALU.subtract)  # E1@3
                ve.tensor_tensor(pl(COL, 2), pl(PRD, 1), pl(PRD, 4), ALU.subtract)  # E5@2

                # weights: UW = w01 - w2 ; W2 = w2   (SGA = A^2 sig^2 = w)
                ve.tensor_tensor(UW[:].rearrange("p (k f) -> p k f", f=F),
                                 pl(SGA, 0, 2), bk(SGA, 2, 2), ALU.subtract)
                ve.tensor_copy(W2[:], pl(SGA, 2))

                # G_j = c_j c_j^T: diag via ACT square, crosses on DVE
                for j in range(2):
                    act.activation(pl(G, 6 * j, 3), pl(COL, 3 * j, 3), ACTF.Square)
                    ve.tensor_tensor(pl(G, 6 * j + 3, 2), bk(COL, 3 * j, 2),
                                     pl(COL, 3 * j + 1, 2), ALU.mult)
                    ve.tensor_tensor(pl(G, 6 * j + 5), pl(COL, 3 * j + 1),
                                     pl(COL, 3 * j + 2), ALU.mult)

                if prev is not None:
                    emit_tail(prev)
                prev = (G, UW, W2, OUT, t)

            emit_tail(prev)
    _split_sync_waits(nc)
    return nc


_NC_CACHE = {}


def get_nc(F=F_PTS, T=T_TILES):
    key = (F, T)
    if key not in _NC_CACHE:
        _NC_CACHE[key] = build_nc(F, T)
    return _NC_CACHE[key]


def prep_in_maps(scaling: np.ndarray, rotation: np.ndarray):
    """Host-side: cast fp16, plane-major [7, Pc] per core (r,z,y,x,s0,s1,s2)."""
    n = scaling.shape[0]
    ntot = N_CORES * P_CORE
    in7 = np.zeros((7, ntot), dtype=np.float16)
    in7[0, :n] = rotation[:, 0]
    in7[1, :n] = rotation[:, 3]
    in7[2, :n] = rotation[:, 2]
    in7[3, :n] = rotation[:, 1]
    in7[0, n:] = 1.0
    in7[4:7, :n] = scaling.T
    return [
        {"in7": np.ascontiguousarray(in7[:, i * P_CORE : (i + 1) * P_CORE])}
        for i in range(N_CORES)
    ]


def assemble_out(results, n):
    out6 = np.concatenate([results[i]["out6"] for i in range(N_CORES)], axis=1)
    out = np.empty((n, 6), dtype=np.float32)
    out[:, 0] = out6[0, :n]
    out[:, 1] = out6[3, :n]
    out[:, 2] = out6[4, :n]
    out[:, 3] = out6[1, :n]
    out[:, 4] = out6[5, :n]
    out[:, 5] = out6[2, :n]
    return out


def kernel(scaling: np.ndarray, rotation: np.ndarray) -> np.ndarray:
    from concourse.bass_utils import run_bass_kernel_spmd

    scaling = np.asarray(scaling, dtype=np.float32)
    rotation = np.asarray(rotation, dtype=np.float32)
    n = scaling.shape[0]
    in_maps = prep_in_maps(scaling, rotation)
    nc = get_nc()
    res = run_bass_kernel_spmd(nc, in_maps, list(range(N_CORES)))
    return assemble_out(res.results, n)
